# revision 1
# baseline (speedup 1.0000x reference)
"""Trainium2 Bass kernel for the OPU (optical matmul + ADC quantize) module.

Math (per r-block of 16 contraction rows, j = k mod 16):
    x_c = x + vmap_lut[j, x+8] = X + vx      (X integer part, vx lut correction)
    w_c = w + wmap_lut[j, w+8] = W + vw
    mm_r = x_c[r] @ w_c[r]                   ([BS,16] @ [16,N])
    adc_r = clip(round(mm_r/16), -128, 127) * 16   (clip can never trigger:
                                                    |mm| <= 16*8.3^2 < 2048)
    out = sum_r adc_r

Implementation:
  - bf16 limb-split matmul: X, W exact ints in bf16; corrections split into
    bf16 limbs vx + vxr (vxr = bf16 residual of vx, ~1e-7 total error).
    One K=96 bf16 matmul per r-block computes the 6 significant terms
    XW + Xvw + Xvwr + vxW + vxvw + vxrW via stacked operands (bf16 streams
    1 cycle/row on PE regardless of K, so extra limbs are free PE-wise).
  - quantize+accumulate fused: f32 accumulator offset by MAGIC = 1.5*2^27,
    where f32 ulp is exactly 16 -> each acc += mm rounds mm to a multiple
    of 16 (RNE), matching jnp.round(mm/16)*16 bit-for-bit.
  - lut corrections via 16 compare-select passes in k-partition layout where
    j = partition%16, so the lut value is a per-partition scalar AP; each
    level's mask feeds two fused scalar_tensor_tensor accumulations.

Sharding: data-parallel over the 2048 token dim (256 tokens/core, 8 cores).
Host prep: reshape+transpose of x to [K, BS] (layout only, no math).
"""
import numpy as np
from contextlib import ExitStack

import concourse.bass as bass
import concourse.bacc as bacc
import concourse.tile as tile
import concourse.mybir as mybir
from concourse import bass_utils

F32 = mybir.dt.float32
BF16 = mybir.dt.bfloat16
FP16 = mybir.dt.float16

B, S, KDIM, N = 2, 1024, 1024, 1024
BS = B * S                  # 2048 tokens
NCORES = 8
TOK = BS // NCORES          # 256 tokens per core
R = KDIM // 16              # 64 blocks
KC = KDIM // 128            # 8 k-chunks of 128 partitions
MC = TOK // 128             # 2 token chunks of 128
EQ = mybir.AluOpType.is_equal
MUL = mybir.AluOpType.mult
SUB = mybir.AluOpType.subtract
ADD = mybir.AluOpType.add

_cache = {}


def _build():
    nc = bacc.Bacc("TRN2", target_bir_lowering=False, debug=False,
                   enable_asserts=False, num_devices=NCORES)
    xt_d = nc.dram_tensor("xt", [KDIM, TOK], F32, kind="ExternalInput").ap()
    w_d = nc.dram_tensor("w", [KDIM, N], F32, kind="ExternalInput").ap()
    vl_d = nc.dram_tensor("vlut", [16, 16], F32, kind="ExternalInput").ap()
    wl_d = nc.dram_tensor("wlut", [16, 16], F32, kind="ExternalInput").ap()
    out_d = nc.dram_tensor("out", [TOK, N], F32, kind="ExternalOutput").ap()

    with tile.TileContext(nc) as tc, ExitStack() as ctx:
        const = ctx.enter_context(tc.tile_pool(name="const", bufs=1))
        raw = ctx.enter_context(tc.tile_pool(name="raw", bufs=2))
        dense = ctx.enter_context(tc.tile_pool(name="dense", bufs=1))
        stk = ctx.enter_context(tc.tile_pool(name="stk", bufs=6))
        tmp = ctx.enter_context(tc.tile_pool(name="tmp", bufs=3))
        psum = ctx.enter_context(tc.tile_pool(name="psum", bufs=3, space="PSUM"))
        accp = ctx.enter_context(tc.tile_pool(name="acc", bufs=1))

        # --- LUTs replicated to 128 partitions: vlut[p, l] = lut[p%16, l]
        vlut = const.tile([128, 16], F32, tag="vlut")
        wlut = const.tile([128, 16], F32, tag="wlut")
        for t, d in ((vlut, vl_d), (wlut, wl_d)):
            for g in range(8):
                nc.sync.dma_start(t[16 * g:16 * (g + 1), :], d[:, :])
        # bf16 residuals of the luts: vres = v - bf16(v)  (for 3rd limbs)
        vlut_b = const.tile([128, 16], BF16, tag="vlutb")
        wlut_b = const.tile([128, 16], BF16, tag="wlutb")
        vres = const.tile([128, 16], F32, tag="vres")
        wres = const.tile([128, 16], F32, tag="wres")
        nc.vector.tensor_copy(vlut_b[:], vlut[:])
        nc.vector.tensor_copy(wlut_b[:], wlut[:])
        nc.vector.tensor_sub(vres[:], vlut[:], vlut_b[:])
        nc.vector.tensor_sub(wres[:], wlut[:], wlut_b[:])

        # --- accumulators: f32 offset by MAGIC = 1.5*2^27 where f32 ulp is
        # exactly 16, so acc += mm rounds mm to a multiple of 16 (RNE).
        MAGIC = float(3 * 2**26)
        accs = []
        for mc in range(MC):
            acc = accp.tile([128, N], F32, tag=f"acc{mc}")
            nc.vector.memset(acc[:], MAGIC)
            accs.append(acc)

        # interleave per k-chunk: construct, stack, matmul, quantize-accum
        for kc in range(KC):
            # load raw f32 chunk of w and xT
            wraw = raw.tile([128, N], F32, tag="wraw")
            nc.sync.dma_start(wraw[:], w_d[128 * kc:128 * (kc + 1), :])
            xraw = raw.tile([128, TOK], F32, tag="xraw")
            nc.sync.dma_start(xraw[:], xt_d[128 * kc:128 * (kc + 1), :])

            # bf16 integer parts
            Wb = dense.tile([128, N], BF16, tag=f"Wb{kc % 2}")
            nc.vector.tensor_copy(Wb[:], wraw[:])
            Xb = dense.tile([128, TOK], BF16, tag=f"Xb{kc % 2}")
            nc.vector.tensor_copy(Xb[:], xraw[:])

            # lut corrections vw, vx (bf16 2nd limbs) + vwr, vxr (3rd limbs)
            vw = dense.tile([128, N], BF16, tag=f"vw{kc % 2}")
            vx = dense.tile([128, TOK], BF16, tag=f"vx{kc % 2}")
            vwr = dense.tile([128, N], BF16, tag=f"vwr{kc % 2}")
            vxr = dense.tile([128, TOK], BF16, tag=f"vxr{kc % 2}")
            for t in (vw, vx, vwr, vxr):
                nc.vector.memset(t[:], 0.0)
            for lvl in range(16):
                c = float(lvl - 8)
                mw = tmp.tile([128, N], BF16, tag="mw")
                nc.vector.tensor_scalar(mw[:], Wb[:], c, None, op0=EQ)
                nc.vector.scalar_tensor_tensor(
                    vw[:], mw[:], wlut[:, lvl:lvl + 1], vw[:], op0=MUL, op1=ADD)
                nc.vector.scalar_tensor_tensor(
                    vwr[:], mw[:], wres[:, lvl:lvl + 1], vwr[:], op0=MUL, op1=ADD)
                mx = tmp.tile([128, TOK], BF16, tag="mx")
                nc.vector.tensor_scalar(mx[:], Xb[:], c, None, op0=EQ)
                nc.vector.scalar_tensor_tensor(
                    vx[:], mx[:], vlut[:, lvl:lvl + 1], vx[:], op0=MUL, op1=ADD)
                nc.vector.scalar_tensor_tensor(
                    vxr[:], mx[:], vres[:, lvl:lvl + 1], vxr[:], op0=MUL, op1=ADD)

            # per r-block: build K=96 stacks via SBUF->SBUF DMA, matmul,
            # quantize+accumulate.  6-term product (X+vx+vxr)(W+vw+vwr)
            # keeping XW, Xvw, Xvwr, vxW, vxvw, vxrW (dropped terms < 1e-5).
            for rb in range(8):
                ps, pe = 16 * rb, 16 * (rb + 1)
                sx = stk.tile([96, TOK], BF16, tag="sx")
                nc.sync.dma_start(sx[0:16, :], Xb[ps:pe, :])
                nc.sync.dma_start(sx[16:32, :], Xb[ps:pe, :])
                nc.sync.dma_start(sx[32:48, :], Xb[ps:pe, :])
                nc.sync.dma_start(sx[48:64, :], vx[ps:pe, :])
                nc.sync.dma_start(sx[64:80, :], vx[ps:pe, :])
                nc.sync.dma_start(sx[80:96, :], vxr[ps:pe, :])
                sw = stk.tile([96, N], BF16, tag="sw")
                nc.sync.dma_start(sw[0:16, :], Wb[ps:pe, :])
                nc.sync.dma_start(sw[16:32, :], vw[ps:pe, :])
                nc.sync.dma_start(sw[32:48, :], vwr[ps:pe, :])
                nc.sync.dma_start(sw[48:64, :], Wb[ps:pe, :])
                nc.sync.dma_start(sw[64:80, :], vw[ps:pe, :])
                nc.sync.dma_start(sw[80:96, :], Wb[ps:pe, :])

                for mc in range(MC):
                    pt = psum.tile([128, N], F32, tag="mm")
                    for h in range(2):
                        nc.tensor.matmul(
                            pt[:, 512 * h:512 * (h + 1)],
                            sx[:, 128 * mc:128 * (mc + 1)],
                            sw[:, 512 * h:512 * (h + 1)],
                            start=True, stop=True)
                    # acc += mm (rounds mm to multiple of 16 via MAGIC offset)
                    nc.vector.tensor_add(accs[mc][:], accs[mc][:], pt[:])

        # --- out = acc - MAGIC (already in output scale), store
        for mc in range(MC):
            o = tmp.tile([128, N], F32, tag="o")
            nc.vector.tensor_scalar(o[:], accs[mc][:], -MAGIC, None, op0=ADD)
            nc.sync.dma_start(out_d[128 * mc:128 * (mc + 1), :], o[:])

    nc.compile()
    return nc


def kernel(input, weight, vmap_lut, wmap_lut):
    if "nc" not in _cache:
        _cache["nc"] = _build()
    nc = _cache["nc"]
    xt = np.ascontiguousarray(
        input.reshape(BS, KDIM).astype(np.float32).T)      # [K, BS]
    w = np.ascontiguousarray(weight.astype(np.float32))
    vl = np.ascontiguousarray(vmap_lut.astype(np.float32))
    wl = np.ascontiguousarray(wmap_lut.astype(np.float32))
    in_maps = [
        {"xt": np.ascontiguousarray(xt[:, TOK * c:TOK * (c + 1)]),
         "w": w, "vlut": vl, "wlut": wl}
        for c in range(NCORES)
    ]
    res = bass_utils.run_bass_kernel_spmd(nc, in_maps, core_ids=list(range(NCORES)))
    out = np.concatenate([res.results[c]["out"] for c in range(NCORES)], axis=0)
    return out.reshape(B, S, N)

